# revision 15
# baseline (speedup 1.0000x reference)
"""Blended-MoE 3-layer MLP (moe_routing) on 8 trn2 NeuronCores.

Math: per layer  z[b,o] = sum_e blend[e,b] * (w[e] @ h[b] + bias[e])[o],
ELU between layers.  Contraction per layer over K = (expert, in-feature):

    z[b,o] = sum_{(e,i)} (blend[e,b] * hT[i,b]) * wT[(e,i), o] + bias-term

The bias term is a K=8 matmul with lhsT = blend itself.  ELU is computed
shifted:  h' = ELU(z)+1 = max(z,0) + min(exp(z),1), with the -1 folded into
the next layer's bias on the host (b_adj = b - w.sum(in_dim)).

Data-parallel across 8 cores (128 batch rows each); expert weights are
replicated, host-side pre-transposed into SBUF-image layout with k-tiles
ordered in matmul-consumption order so the PE chases the DMA stream.
"""

import numpy as np

import bass_rust
import concourse.bass as bass
import concourse.mybir as mybir
import concourse.tile as tile
from concourse.bass_utils import run_bass_kernel_spmd

# ---- config ----------------------------------------------------------------
N_CORES = 8
B, E = 1024, 8
DIN, D1, D2, D3 = 480, 512, 512, 311

F32 = mybir.dt.float32
F16 = mybir.dt.float16
NP16 = np.float16

PROFILE = {"trace": False, "tmpdir": None}
LAST_RESULT = [None]
_NC_CACHE = {}
_SPLIT_N = [0]


def _split_multi_waits(nc, max_waits=1):
    """This container's walrus only supports one sync-wait command per
    instruction; spill extras onto same-engine NOPs inserted just before."""
    for f in nc.m.functions:
        for bb in f.blocks:
            insts = bb.instructions
            i = 0
            while i < len(insts):
                inst = insts[i]
                si = inst.sync_info
                if si is not None and len(si.on_wait) > max_waits:
                    waits = list(si.on_wait)
                    extra, keep = waits[:-max_waits], waits[-max_waits:]
                    for w in extra:
                        _SPLIT_N[0] += 1
                        nop = mybir.InstNoOp(
                            name=f"wsplit-{_SPLIT_N[0]}", ins=[], outs=[]
                        )
                        nop.engine = inst.engine
                        nop.sync_info = bass_rust.SyncInfo(
                            on_wait=[w], on_update=[]
                        )
                        insts.insert(i, nop)
                        i += 1
                    inst.sync_info = bass_rust.SyncInfo(
                        on_wait=keep, on_update=list(si.on_update)
                    )
                i += 1


def _patch_minimal_tail():
    """Tile's kernel-tail is drain + 2 full all-engine barriers + sem clear
    (~10us).  Replace with drain + one barrier + range clear."""
    from concourse.vector_clock import ScopedClock

    if getattr(tile.TileContext, "_min_tail_patched", False):
        return

    def _drain_and_barrier(self, tick_clock, wait_clock):
        nc = self.nc
        drain_inst = nc.sync.drain()
        wait_clock.add_sem_waits(
            drain_inst.ins, ScopedClock({None: tick_clock.global_clock})
        )
        nc.all_engine_barrier()
        popped = nc._tile_sem_poison_stack.pop()
        assert popped is self._sem_poison
        assert self.sems is not None
        nc.clear_and_free_semaphores(list(self.sems.allocated().values()))
        # original ends with a second all_engine_barrier; the gpsimd range
        # clear is the last thing this engine does and the next NEFF
        # execution starts only after every engine ended, so skip it.

    tile.TileContext._drain_and_barrier = _drain_and_barrier
    tile.TileContext._min_tail_patched = True


_patch_minimal_tail()


# Per-layer k-tile plans: (n_out, n_full_ktiles) ; w0 has 24 full + 8 of K=96
N_L = (D1, D2, D3)


def _bcast_e(ap_tile, cols):
    """[128, cols] AP -> [128, 8, cols] with stride-0 expert dim."""
    return ap_tile.unsqueeze(1).broadcast_to([ap_tile.shape[0], E, cols])


def _build_nc():
    nc = bass.Bass()

    # DRAM inputs (shared weight images + per-core misc)
    w0a_d = nc.dram_tensor("w0a", [128, 24 * D1], F16, kind="ExternalInput")
    w0b_d = nc.dram_tensor("w0b", [96, 8 * D1], F16, kind="ExternalInput")
    w1_d = nc.dram_tensor("w1s", [128, 32 * D2], F16, kind="ExternalInput")
    w2_d = nc.dram_tensor("w2s", [128, 32 * D3], F16, kind="ExternalInput")
    # misc128 = [xT(512) | bb(1024) | ident(128)]
    MISC_COLS = 512 + 1024 + 128
    misc_d = nc.dram_tensor("misc", [128, MISC_COLS], F16, kind="ExternalInput")
    # misc8 = [aug(128) | waug0/1/2] on 8 partitions
    M8_COLS = 128 + D1 + D2 + D3
    misc8_d = nc.dram_tensor("misc8", [8, M8_COLS], F16, kind="ExternalInput")
    out_d = nc.dram_tensor("out", [128, D3], F16, kind="ExternalOutput")

    with tile.TileContext(nc) as tc:
        with (
            tc.tile_pool(name="const", bufs=1) as const,
            tc.tile_pool(name="w", bufs=1) as wpool,
            tc.tile_pool(name="acts", bufs=2) as acts,
            tc.tile_pool(name="tmp", bufs=2) as tmp,
            tc.tile_pool(name="zp", bufs=2, space="PSUM") as zp,
            tc.tile_pool(name="tp", bufs=2, space="PSUM") as tp,
        ):
            # ---- PE warm-up: dummy matmuls so HAM un-throttles to 2.4GHz
            # while the weight DMAs stream in ----
            scratch = const.tile([128, 512], F16)
            nc.vector.memset(scratch[:], 0.0)
            zwarm = tp.tile([128, 512], F32, tag="warm", bufs=1)

            def filler(n=1):
                for _ in range(n):
                    nc.tensor.matmul(
                        zwarm[:], scratch[:, 0:128], scratch[:],
                        start=True, stop=True,
                    )

            filler(12)

            # ---- DMA issue: misc + first w0a group on scalar queue,
            # remaining weights on gpsimd queue, out on sync+scalar ----
            misc_sb = const.tile([128, MISC_COLS], F16)
            nc.scalar.dma_start(misc_sb[:], misc_d[:])
            misc8_sb = const.tile([8, M8_COLS], F16)
            nc.scalar.dma_start(misc8_sb[:], misc8_d[:])
            # keep the sync DGE warm so the final out DMA has low latency
            syncwarm = const.tile([128, 8], F16)
            nc.sync.dma_start(syncwarm[:], misc_d[:, 0:8])
            w0a_sb = wpool.tile([128, 24 * D1], F16, tag="w0a")

            xt_sb = misc_sb[:, 0:512]
            bb_sb = misc_sb[:, 512:1536]
            ident = misc_sb[:, 1536:1664]
            aug_sb = misc8_sb[:, 0:128]
            waug = [
                misc8_sb[:, 128 : 128 + D1],
                misc8_sb[:, 128 + D1 : 128 + D1 + D2],
                misc8_sb[:, 128 + D1 + D2 : 128 + D1 + D2 + D3],
            ]

            # weight tiles; groups of 8 k-tiles per DMA
            for g in range(0, 3):
                nc.gpsimd.dma_start(
                    w0a_sb[:, g * 8 * D1 : (g + 1) * 8 * D1],
                    w0a_d[:, g * 8 * D1 : (g + 1) * 8 * D1],
                )
            w0b_sb = wpool.tile([96, 8 * D1], F16, tag="w0b")
            nc.gpsimd.dma_start(w0b_sb[:], w0b_d[:])
            w1_sb = wpool.tile([128, 32 * D2], F16, tag="w1")
            for g in range(4):
                nc.gpsimd.dma_start(
                    w1_sb[:, g * 8 * D2 : (g + 1) * 8 * D2],
                    w1_d[:, g * 8 * D2 : (g + 1) * 8 * D2],
                )
            w2_sb = wpool.tile([128, 32 * D3], F16, tag="w2")
            for lo, hi in ((0, 8), (8, 16), (16, 24), (24, 28), (28, 32)):
                nc.gpsimd.dma_start(
                    w2_sb[:, lo * D3 : hi * D3],
                    w2_d[:, lo * D3 : hi * D3],
                )

            # ---- layer 0: he expansion from xT ----
            # he layout: column block (it*8+e)*128 holds he(e, it-tile)
            he0 = acts.tile([128, 32 * 128], F16, tag="he")
            for it in range(4):
                base = it * 1024
                xslice = xt_sb[:, it * 128 : (it + 1) * 128]
                if it == 0:
                    # split: e=0 first so the first matmul starts early
                    nc.vector.tensor_tensor(
                        he0[:, 0:128], xslice[:], bb_sb[:, 0:128],
                        mybir.AluOpType.mult,
                    )
                    nc.vector.tensor_tensor(
                        he0[:, 128:1024].rearrange("p (e c) -> p e c", e=7),
                        _bcast_e(xslice, 128)[:, 1:8],
                        bb_sb[:, 128:1024].rearrange("p (e c) -> p e c", e=7),
                        mybir.AluOpType.mult,
                    )
                else:
                    nc.vector.tensor_tensor(
                        he0[:, base : base + 1024].rearrange(
                            "p (e c) -> p e c", e=8
                        ),
                        _bcast_e(xslice, 128),
                        bb_sb[:].rearrange("p (e c) -> p e c", e=8),
                        mybir.AluOpType.mult,
                    )

            src_he = he0
            for l in range(3):
                n = N_L[l]
                z = zp.tile([128, n], F32, tag="z")
                # bias via K=8 matmul: lhsT = blend [8,128], rhs = bias [8,n]
                nc.tensor.matmul(
                    z[:], aug_sb[:], waug[l][:, :n], start=True, stop=False
                )
                # main contraction, it-major k-tile order
                if l == 0:
                    for it in range(3):
                        for e in range(E):
                            kt = it * 8 + e
                            nc.tensor.matmul(
                                z[:],
                                src_he[:, kt * 128 : (kt + 1) * 128],
                                w0a_sb[:, kt * n : (kt + 1) * n],
                                start=False, stop=False,
                            )
                        filler(3)
                    for e in range(E):
                        kt = 24 + e
                        nc.tensor.matmul(
                            z[:],
                            src_he[0:96, kt * 128 : (kt + 1) * 128],
                            w0b_sb[:, e * n : (e + 1) * n],
                            start=False, stop=(e == 7),
                        )
                    filler(3)
                else:
                    wsb = w1_sb if l == 1 else w2_sb
                    for it in range(4):
                        for e in range(E):
                            kt = it * 8 + e
                            nc.tensor.matmul(
                                z[:],
                                src_he[:, kt * 128 : (kt + 1) * 128],
                                wsb[:, kt * n : (kt + 1) * n],
                                start=False, stop=(kt == 31),
                            )
                        if l == 1:
                            filler(3)

                if l == 2:
                    out_sb = tmp.tile([128, D3], F16, tag="osb")
                    H1 = 156
                    nc.scalar.copy(out_sb[:, :H1], z[:, :H1])
                    nc.vector.tensor_scalar(
                        out_sb[:, H1:], z[:, H1:], 0.0, None,
                        mybir.AluOpType.add,
                    )
                    nc.sync.dma_start(out_d[:], out_sb[:])
                    break

                # ---- boundary: h' = max(z,0) + min(exp(z),1), per 128-col
                # tile so the pipeline restarts the PE quickly ----
                ex = tmp.tile([128, n], F32, tag="ex")
                p = tmp.tile([128, n], F32, tag="p")
                h = tmp.tile([128, n], F16, tag="h")
                tps = tp.tile([128, 4 * 128], F16, tag="tps")
                he = acts.tile([128, 32 * 128], F16, tag="he")
                for it in range(4):
                    sl = slice(it * 128, (it + 1) * 128)
                    nc.scalar.activation(
                        ex[:, sl], z[:, sl], mybir.ActivationFunctionType.Exp
                    )
                    nc.vector.tensor_scalar(
                        p[:, sl], z[:, sl], 0.0, None, mybir.AluOpType.max
                    )
                    nc.vector.scalar_tensor_tensor(
                        h[:, sl], ex[:, sl], 1.0, p[:, sl],
                        mybir.AluOpType.min, mybir.AluOpType.add,
                    )
                    nc.tensor.transpose(tps[:, sl], h[:, sl], ident[:])
                    filler(1)
                    if l == 1 and it == 0:
                        nc.sync.dma_start(syncwarm[:], h[:, 0:8])
                    base = it * 1024
                    tslice = tps[:, sl]
                    if it == 0:
                        nc.vector.tensor_tensor(
                            he[:, 0:128], tslice[:], bb_sb[:, 0:128],
                            mybir.AluOpType.mult,
                        )
                        nc.vector.tensor_tensor(
                            he[:, 128:1024].rearrange("p (e c) -> p e c", e=7),
                            _bcast_e(tslice, 128)[:, 1:8],
                            bb_sb[:, 128:1024].rearrange(
                                "p (e c) -> p e c", e=7
                            ),
                            mybir.AluOpType.mult,
                        )
                    else:
                        nc.vector.tensor_tensor(
                            he[:, base : base + 1024].rearrange(
                                "p (e c) -> p e c", e=8
                            ),
                            _bcast_e(tslice, 128),
                            bb_sb[:].rearrange("p (e c) -> p e c", e=8),
                            mybir.AluOpType.mult,
                        )
                src_he = he

    _split_multi_waits(nc)
    return nc


# ---- host-side packing -----------------------------------------------------


def _ktile_img(w, n_kt, kt_of):
    """Pack k-tiles of wT into a [rows, n_kt*dout] image.

    kt_of(k) -> (expert, feat_lo, feat_hi); rows = max tile height."""
    dout = w.shape[1]
    rows = max(hi - lo for _, lo, hi in map(kt_of, range(n_kt)))
    img = np.zeros((rows, n_kt * dout), np.float32)
    for k in range(n_kt):
        e, lo, hi = kt_of(k)
        img[0 : hi - lo, k * dout : (k + 1) * dout] = w[e, :, lo:hi].T
    return np.ascontiguousarray(img).astype(NP16)


def kernel(x, weight_blend, w0, b0, w1, b1, w2, b2):
    if "nc" not in _NC_CACHE:
        _NC_CACHE["nc"] = _build_nc()
    nc = _NC_CACHE["nc"]

    x = np.asarray(x, np.float32)
    weight_blend = np.asarray(weight_blend, np.float32)
    w0, b0 = np.asarray(w0, np.float32), np.asarray(b0, np.float32)
    w1, b1 = np.asarray(w1, np.float32), np.asarray(b1, np.float32)
    w2, b2 = np.asarray(w2, np.float32), np.asarray(b2, np.float32)

    # fp16-rounded weights for exact -1-shift bias folding
    w1h = np.float32(NP16(w1))
    w2h = np.float32(NP16(w2))
    b1a = b1 - w1h.sum(axis=2)
    b2a = b2 - w2h.sum(axis=2)

    ident = np.eye(128, dtype=np.float32)

    bc = B // N_CORES
    in_maps = []
    for c in range(N_CORES):
        # expert rotation per core: slot s holds expert (s+c)%8, spreading
        # the 8 cores' simultaneous HBM reads across different regions
        perm = [(s + c) % E for s in range(E)]
        w0r, w1r, w2r = w0[perm], w1[perm], w2[perm]
        w0a = _ktile_img(
            w0r, 24, lambda k: (k % 8, (k // 8) * 128, (k // 8) * 128 + 128)
        )
        w0b = _ktile_img(w0r, 8, lambda k: (k, 384, 480))
        w1img = _ktile_img(
            w1r, 32, lambda k: (k % 8, (k // 8) * 128, (k // 8) * 128 + 128)
        )
        w2img = _ktile_img(
            w2r, 32, lambda k: (k % 8, (k // 8) * 128, (k // 8) * 128 + 128)
        )

        sl = slice(c * bc, (c + 1) * bc)
        xT = np.zeros((4 * 128, bc), np.float32)
        xT[:DIN] = x[sl].T
        xt_img = xT.reshape(4, 128, bc).transpose(1, 0, 2).reshape(128, 4 * bc)
        bl = weight_blend[perm][:, sl]  # (8, 128), expert-rotated
        bb_img = np.broadcast_to(bl[None], (128, E, bc)).reshape(128, E * bc)
        misc = np.concatenate([xt_img, bb_img, ident], axis=1).astype(NP16)
        misc8 = np.concatenate(
            [bl, b0[perm], b1a[perm], b2a[perm]], axis=1
        ).astype(NP16)
        in_maps.append(
            {
                "w0a": w0a,
                "w0b": w0b,
                "w1s": w1img,
                "w2s": w2img,
                "misc": np.ascontiguousarray(misc),
                "misc8": np.ascontiguousarray(misc8),
            }
        )

    res = run_bass_kernel_spmd(
        nc,
        in_maps,
        core_ids=list(range(N_CORES)),
        trace=PROFILE["trace"],
        tmpdir=PROFILE["tmpdir"],
    )
    LAST_RESULT[0] = res
    return np.concatenate(
        [res.results[c]["out"] for c in range(N_CORES)], axis=0
    ).astype(np.float32)


# revision 16
# speedup vs baseline: 1.0138x; 1.0138x over previous
"""Blended-MoE 3-layer MLP (moe_routing) on 8 trn2 NeuronCores.

Math: per layer  z[b,o] = sum_e blend[e,b] * (w[e] @ h[b] + bias[e])[o],
ELU between layers.  Contraction per layer over K = (expert, in-feature):

    z[b,o] = sum_{(e,i)} (blend[e,b] * hT[i,b]) * wT[(e,i), o] + bias-term

The bias term is a K=8 matmul with lhsT = blend itself.  ELU is computed
shifted:  h' = ELU(z)+1 = max(z,0) + min(exp(z),1), with the -1 folded into
the next layer's bias on the host (b_adj = b - w.sum(in_dim)).

Data-parallel across 8 cores (128 batch rows each); expert weights are
replicated, host-side pre-transposed into SBUF-image layout with k-tiles
ordered in matmul-consumption order so the PE chases the DMA stream.
"""

import numpy as np

import bass_rust
import concourse.bass as bass
import concourse.mybir as mybir
import concourse.tile as tile
from concourse.bass_utils import run_bass_kernel_spmd

# ---- config ----------------------------------------------------------------
N_CORES = 8
B, E = 1024, 8
DIN, D1, D2, D3 = 480, 512, 512, 311

F32 = mybir.dt.float32
F16 = mybir.dt.float16
NP16 = np.float16

PROFILE = {"trace": False, "tmpdir": None}
LAST_RESULT = [None]
_NC_CACHE = {}
_SPLIT_N = [0]


def _split_multi_waits(nc, max_waits=1):
    """This container's walrus only supports one sync-wait command per
    instruction; spill extras onto same-engine NOPs inserted just before."""
    for f in nc.m.functions:
        for bb in f.blocks:
            insts = bb.instructions
            i = 0
            while i < len(insts):
                inst = insts[i]
                si = inst.sync_info
                if si is not None and len(si.on_wait) > max_waits:
                    waits = list(si.on_wait)
                    extra, keep = waits[:-max_waits], waits[-max_waits:]
                    for w in extra:
                        _SPLIT_N[0] += 1
                        nop = mybir.InstNoOp(
                            name=f"wsplit-{_SPLIT_N[0]}", ins=[], outs=[]
                        )
                        nop.engine = inst.engine
                        nop.sync_info = bass_rust.SyncInfo(
                            on_wait=[w], on_update=[]
                        )
                        insts.insert(i, nop)
                        i += 1
                    inst.sync_info = bass_rust.SyncInfo(
                        on_wait=keep, on_update=list(si.on_update)
                    )
                i += 1


def _patch_minimal_tail():
    """Tile's kernel-tail is drain + 2 full all-engine barriers + sem clear
    (~10us).  Replace with drain + one barrier + range clear."""
    from concourse.vector_clock import ScopedClock

    if getattr(tile.TileContext, "_min_tail_patched", False):
        return

    def _drain_and_barrier(self, tick_clock, wait_clock):
        nc = self.nc
        drain_inst = nc.sync.drain()
        wait_clock.add_sem_waits(
            drain_inst.ins, ScopedClock({None: tick_clock.global_clock})
        )
        nc.all_engine_barrier()
        popped = nc._tile_sem_poison_stack.pop()
        assert popped is self._sem_poison
        assert self.sems is not None
        nc.clear_and_free_semaphores(list(self.sems.allocated().values()))
        # original ends with a second all_engine_barrier; the gpsimd range
        # clear is the last thing this engine does and the next NEFF
        # execution starts only after every engine ended, so skip it.

    tile.TileContext._drain_and_barrier = _drain_and_barrier
    tile.TileContext._min_tail_patched = True


_patch_minimal_tail()


# Per-layer k-tile plans: (n_out, n_full_ktiles) ; w0 has 24 full + 8 of K=96
N_L = (D1, D2, D3)


def _bcast_e(ap_tile, cols):
    """[128, cols] AP -> [128, 8, cols] with stride-0 expert dim."""
    return ap_tile.unsqueeze(1).broadcast_to([ap_tile.shape[0], E, cols])


def _build_nc():
    nc = bass.Bass()

    # DRAM inputs (shared weight images + per-core misc)
    w0a_d = nc.dram_tensor("w0a", [128, 24 * D1], F16, kind="ExternalInput")
    w0b_d = nc.dram_tensor("w0b", [96, 8 * D1], F16, kind="ExternalInput")
    w1_d = nc.dram_tensor("w1s", [128, 32 * D2], F16, kind="ExternalInput")
    w2_d = nc.dram_tensor("w2s", [128, 32 * D3], F16, kind="ExternalInput")
    # misc128 = [xT(512) | bb(1024) | ident(128)]
    MISC_COLS = 512 + 1024 + 128
    misc_d = nc.dram_tensor("misc", [128, MISC_COLS], F16, kind="ExternalInput")
    # misc8 = [aug(128) | waug0/1/2] on 8 partitions
    M8_COLS = 128 + D1 + D2 + D3
    misc8_d = nc.dram_tensor("misc8", [8, M8_COLS], F16, kind="ExternalInput")
    out_d = nc.dram_tensor("out", [128, D3], F16, kind="ExternalOutput")

    with tile.TileContext(nc) as tc:
        with (
            tc.tile_pool(name="const", bufs=1) as const,
            tc.tile_pool(name="w", bufs=1) as wpool,
            tc.tile_pool(name="acts", bufs=2) as acts,
            tc.tile_pool(name="tmp", bufs=2) as tmp,
            tc.tile_pool(name="zp", bufs=2, space="PSUM") as zp,
            tc.tile_pool(name="tp", bufs=2, space="PSUM") as tp,
        ):
            # ---- PE warm-up: dummy matmuls so HAM un-throttles to 2.4GHz
            # while the weight DMAs stream in ----
            scratch = const.tile([128, 512], F16)
            nc.vector.memset(scratch[:], 0.0)
            zwarm = tp.tile([128, 512], F32, tag="warm", bufs=1)

            def filler(n=1):
                for _ in range(n):
                    nc.tensor.matmul(
                        zwarm[:], scratch[:, 0:128], scratch[:],
                        start=True, stop=True,
                    )

            filler(12)

            # ---- DMA issue: misc + first w0a group on scalar queue,
            # remaining weights on gpsimd queue, out on sync+scalar ----
            misc_sb = const.tile([128, MISC_COLS], F16)
            nc.scalar.dma_start(misc_sb[:], misc_d[:])
            misc8_sb = const.tile([8, M8_COLS], F16)
            nc.scalar.dma_start(misc8_sb[:], misc8_d[:])
            # keep the sync DGE warm so the final out DMA has low latency
            syncwarm = const.tile([128, 8], F16)
            nc.sync.dma_start(syncwarm[:], misc_d[:, 0:8])
            w0a_sb = wpool.tile([128, 24 * D1], F16, tag="w0a")

            xt_sb = misc_sb[:, 0:512]
            bb_sb = misc_sb[:, 512:1536]
            ident = misc_sb[:, 1536:1664]
            aug_sb = misc8_sb[:, 0:128]
            waug = [
                misc8_sb[:, 128 : 128 + D1],
                misc8_sb[:, 128 + D1 : 128 + D1 + D2],
                misc8_sb[:, 128 + D1 + D2 : 128 + D1 + D2 + D3],
            ]

            # weight tiles; groups of 8 k-tiles per DMA
            for g in range(0, 3):
                nc.gpsimd.dma_start(
                    w0a_sb[:, g * 8 * D1 : (g + 1) * 8 * D1],
                    w0a_d[:, g * 8 * D1 : (g + 1) * 8 * D1],
                )
            w0b_sb = wpool.tile([96, 8 * D1], F16, tag="w0b")
            nc.gpsimd.dma_start(w0b_sb[:], w0b_d[:])
            w1_sb = wpool.tile([128, 32 * D2], F16, tag="w1")
            for g in range(4):
                nc.gpsimd.dma_start(
                    w1_sb[:, g * 8 * D2 : (g + 1) * 8 * D2],
                    w1_d[:, g * 8 * D2 : (g + 1) * 8 * D2],
                )
            w2_sb = wpool.tile([128, 32 * D3], F16, tag="w2")
            for lo, hi in ((0, 8), (8, 16), (16, 24), (24, 28), (28, 32)):
                nc.gpsimd.dma_start(
                    w2_sb[:, lo * D3 : hi * D3],
                    w2_d[:, lo * D3 : hi * D3],
                )

            # ---- layer 0: he expansion from xT ----
            # he layout: column block (it*8+e)*128 holds he(e, it-tile)
            he0 = acts.tile([128, 32 * 128], F16, tag="he")
            for it in range(4):
                base = it * 1024
                xslice = xt_sb[:, it * 128 : (it + 1) * 128]
                if it == 0:
                    # split: e=0 first so the first matmul starts early
                    nc.vector.tensor_tensor(
                        he0[:, 0:128], xslice[:], bb_sb[:, 0:128],
                        mybir.AluOpType.mult,
                    )
                    nc.vector.tensor_tensor(
                        he0[:, 128:1024].rearrange("p (e c) -> p e c", e=7),
                        _bcast_e(xslice, 128)[:, 1:8],
                        bb_sb[:, 128:1024].rearrange("p (e c) -> p e c", e=7),
                        mybir.AluOpType.mult,
                    )
                else:
                    nc.vector.tensor_tensor(
                        he0[:, base : base + 1024].rearrange(
                            "p (e c) -> p e c", e=8
                        ),
                        _bcast_e(xslice, 128),
                        bb_sb[:].rearrange("p (e c) -> p e c", e=8),
                        mybir.AluOpType.mult,
                    )

            src_he = he0
            for l in range(3):
                n = N_L[l]
                z = zp.tile([128, n], F32, tag="z")
                # bias via K=8 matmul: lhsT = blend [8,128], rhs = bias [8,n]
                nc.tensor.matmul(
                    z[:], aug_sb[:], waug[l][:, :n], start=True, stop=False
                )
                # main contraction, it-major k-tile order
                if l == 0:
                    for it in range(3):
                        for e in range(E):
                            kt = it * 8 + e
                            nc.tensor.matmul(
                                z[:],
                                src_he[:, kt * 128 : (kt + 1) * 128],
                                w0a_sb[:, kt * n : (kt + 1) * n],
                                start=False, stop=False,
                            )
                        filler(3)
                    for e in range(E):
                        kt = 24 + e
                        nc.tensor.matmul(
                            z[:],
                            src_he[0:96, kt * 128 : (kt + 1) * 128],
                            w0b_sb[:, e * n : (e + 1) * n],
                            start=False, stop=(e == 7),
                        )
                    filler(3)
                else:
                    wsb = w1_sb if l == 1 else w2_sb
                    for it in range(4):
                        for e in range(E):
                            kt = it * 8 + e
                            nc.tensor.matmul(
                                z[:],
                                src_he[:, kt * 128 : (kt + 1) * 128],
                                wsb[:, kt * n : (kt + 1) * n],
                                start=False, stop=(kt == 31),
                            )
                        if l == 1:
                            filler(3)

                if l == 2:
                    out_sb = tmp.tile([128, D3], F16, tag="osb")
                    nc.scalar.copy(out_sb[:], z[:])
                    nc.sync.dma_start(out_d[:], out_sb[:])
                    break

                # ---- boundary: h' = max(z,0) + min(exp(z),1), per 128-col
                # tile so the pipeline restarts the PE quickly ----
                ex = tmp.tile([128, n], F32, tag="ex")
                p = tmp.tile([128, n], F32, tag="p")
                h = tmp.tile([128, n], F16, tag="h")
                tps = tp.tile([128, 4 * 128], F16, tag="tps")
                he = acts.tile([128, 32 * 128], F16, tag="he")
                for it in range(4):
                    sl = slice(it * 128, (it + 1) * 128)
                    nc.scalar.activation(
                        ex[:, sl], z[:, sl], mybir.ActivationFunctionType.Exp
                    )
                    nc.vector.tensor_scalar(
                        p[:, sl], z[:, sl], 0.0, None, mybir.AluOpType.max
                    )
                    nc.vector.scalar_tensor_tensor(
                        h[:, sl], ex[:, sl], 1.0, p[:, sl],
                        mybir.AluOpType.min, mybir.AluOpType.add,
                    )
                    nc.tensor.transpose(tps[:, sl], h[:, sl], ident[:])
                    filler(1)
                    if l == 1 and it == 0:
                        nc.sync.dma_start(syncwarm[:], h[:, 0:8])
                    base = it * 1024
                    tslice = tps[:, sl]
                    if it == 0:
                        nc.vector.tensor_tensor(
                            he[:, 0:128], tslice[:], bb_sb[:, 0:128],
                            mybir.AluOpType.mult,
                        )
                        nc.vector.tensor_tensor(
                            he[:, 128:1024].rearrange("p (e c) -> p e c", e=7),
                            _bcast_e(tslice, 128)[:, 1:8],
                            bb_sb[:, 128:1024].rearrange(
                                "p (e c) -> p e c", e=7
                            ),
                            mybir.AluOpType.mult,
                        )
                    else:
                        nc.vector.tensor_tensor(
                            he[:, base : base + 1024].rearrange(
                                "p (e c) -> p e c", e=8
                            ),
                            _bcast_e(tslice, 128),
                            bb_sb[:].rearrange("p (e c) -> p e c", e=8),
                            mybir.AluOpType.mult,
                        )
                src_he = he

    _split_multi_waits(nc)
    return nc


# ---- host-side packing -----------------------------------------------------


def _ktile_img(w, n_kt, kt_of):
    """Pack k-tiles of wT into a [rows, n_kt*dout] image.

    kt_of(k) -> (expert, feat_lo, feat_hi); rows = max tile height."""
    dout = w.shape[1]
    rows = max(hi - lo for _, lo, hi in map(kt_of, range(n_kt)))
    img = np.zeros((rows, n_kt * dout), np.float32)
    for k in range(n_kt):
        e, lo, hi = kt_of(k)
        img[0 : hi - lo, k * dout : (k + 1) * dout] = w[e, :, lo:hi].T
    return np.ascontiguousarray(img).astype(NP16)


def kernel(x, weight_blend, w0, b0, w1, b1, w2, b2):
    if "nc" not in _NC_CACHE:
        _NC_CACHE["nc"] = _build_nc()
    nc = _NC_CACHE["nc"]

    x = np.asarray(x, np.float32)
    weight_blend = np.asarray(weight_blend, np.float32)
    w0, b0 = np.asarray(w0, np.float32), np.asarray(b0, np.float32)
    w1, b1 = np.asarray(w1, np.float32), np.asarray(b1, np.float32)
    w2, b2 = np.asarray(w2, np.float32), np.asarray(b2, np.float32)

    # fp16-rounded weights for exact -1-shift bias folding
    w1h = np.float32(NP16(w1))
    w2h = np.float32(NP16(w2))
    b1a = b1 - w1h.sum(axis=2)
    b2a = b2 - w2h.sum(axis=2)

    ident = np.eye(128, dtype=np.float32)

    bc = B // N_CORES
    in_maps = []
    for c in range(N_CORES):
        # expert rotation per core: slot s holds expert (s+c)%8, spreading
        # the 8 cores' simultaneous HBM reads across different regions
        perm = [(s + c) % E for s in range(E)]
        w0r, w1r, w2r = w0[perm], w1[perm], w2[perm]
        w0a = _ktile_img(
            w0r, 24, lambda k: (k % 8, (k // 8) * 128, (k // 8) * 128 + 128)
        )
        w0b = _ktile_img(w0r, 8, lambda k: (k, 384, 480))
        w1img = _ktile_img(
            w1r, 32, lambda k: (k % 8, (k // 8) * 128, (k // 8) * 128 + 128)
        )
        w2img = _ktile_img(
            w2r, 32, lambda k: (k % 8, (k // 8) * 128, (k // 8) * 128 + 128)
        )

        sl = slice(c * bc, (c + 1) * bc)
        xT = np.zeros((4 * 128, bc), np.float32)
        xT[:DIN] = x[sl].T
        xt_img = xT.reshape(4, 128, bc).transpose(1, 0, 2).reshape(128, 4 * bc)
        bl = weight_blend[perm][:, sl]  # (8, 128), expert-rotated
        bb_img = np.broadcast_to(bl[None], (128, E, bc)).reshape(128, E * bc)
        misc = np.concatenate([xt_img, bb_img, ident], axis=1).astype(NP16)
        misc8 = np.concatenate(
            [bl, b0[perm], b1a[perm], b2a[perm]], axis=1
        ).astype(NP16)
        in_maps.append(
            {
                "w0a": w0a,
                "w0b": w0b,
                "w1s": w1img,
                "w2s": w2img,
                "misc": np.ascontiguousarray(misc),
                "misc8": np.ascontiguousarray(misc8),
            }
        )

    res = run_bass_kernel_spmd(
        nc,
        in_maps,
        core_ids=list(range(N_CORES)),
        trace=PROFILE["trace"],
        tmpdir=PROFILE["tmpdir"],
    )
    LAST_RESULT[0] = res
    return np.concatenate(
        [res.results[c]["out"] for c in range(N_CORES)], axis=0
    ).astype(np.float32)
